# revision 4
# baseline (speedup 1.0000x reference)
"""BiLSTM on 8 TRN2 cores — step B: 8-way gate-split recurrence with per-step
cross-core h all-gather via remote_dma_broadcast.  Raw bass (no Tile).

Sharding: every core runs BOTH directions.  Core r owns H-dims
[128r, 128r+128) of both directions: it computes that slice of all four
gates (host reorders gate rows to [i|f|o|g~] so sigmoid is one contiguous
span), updates c/h for its 128 dims, and broadcasts its h^T chunk [128, 64]
bf16 to all 8 cores each step.  The two directions ping-pong so the
broadcast of one direction hides under the compute of the other.

Phase 1 (per direction): xg = x @ W_ih_slice^T + bias_slice, a plain GEMM
(x^T tiles via DMA-transpose of host-cast bf16 x), xg stored time-major in
DRAM scratch.  Phase 2: the recurrence.
"""

import sys
import time

import numpy as np
import ml_dtypes

sys.path.insert(0, "/opt/trn_rl_repo")

import concourse.bass as bass
import concourse.mybir as mybir
from concourse import bacc
from concourse.bass import ds, ts
from concourse.bass_utils import run_bass_kernel_spmd

F32 = mybir.dt.float32
BF16 = mybir.dt.bfloat16
AF = mybir.ActivationFunctionType
OP = mybir.AluOpType
BF16_NP = ml_dtypes.bfloat16

B, S_FULL, I_IN, H = 64, 512, 1024, 1024
NSL = 512            # gate slice per core (128 of each gate)
HSL = 128            # h dims per core
NCORES = 8


def build(S=S_FULL, sim=False):
    KI = I_IN // 128   # 8
    KH = H // 128      # 8
    TCH = S // 128     # s-quarters per b row in phase 1
    NCH = B * TCH      # chunks per direction in phase 1

    nc = bacc.Bacc("TRN2", target_bir_lowering=False, debug=False,
                   num_devices=NCORES)

    # ---- DRAM ----
    x_d = {}
    wihT_d = {}
    whhT_d = {}
    bias_d = {}
    hout_d = {}
    xg_d = {}
    for d in "fb":
        x_d[d] = nc.dram_tensor(f"x{d}", [B, S, I_IN], BF16, kind="ExternalInput")
        wihT_d[d] = nc.dram_tensor(f"wihT{d}", [I_IN, NSL], BF16, kind="ExternalInput")
        whhT_d[d] = nc.dram_tensor(f"whhT{d}", [H, NSL], BF16, kind="ExternalInput")
        bias_d[d] = nc.dram_tensor(f"bias{d}", [1, NSL], BF16, kind="ExternalInput")
        hout_d[d] = nc.dram_tensor(f"h{d}", [B, S, HSL], F32, kind="ExternalOutput")
        xg_d[d] = nc.dram_tensor(f"xg{d}", [S * B, NSL], BF16, kind="Internal")

    # ---- semaphores ----
    sem = {}
    def SEM(name):
        sem[name] = nc.alloc_semaphore(name)
        return sem[name]
    for d in "fb":
        for nm in ("mm", "add", "act", "c", "tc", "h", "T", "cast", "prep"):
            SEM(f"{nm}_{d}")
        for p in range(2):
            SEM(f"r_{d}{p}"); SEM(f"l_{d}{p}"); SEM(f"shd_{d}{p}")
        for m in range(3):
            SEM(f"sxg_{d}{m}")
    for nm in ("sxT0", "sxT1", "sxT2", "sxT3", "mm1", "evac1", "p1out", "sw",
               "initv", "initg"):
        SEM(nm)

    # ---- SBUF persistent ----
    sb = nc.alloc_sbuf_tensor
    whhT_sb = {d: sb(f"whhT_sb{d}", [128, KH * NSL], BF16).ap() for d in "fb"}
    wihT_sb = {d: sb(f"wihT_sb{d}", [128, KI * NSL], BF16).ap() for d in "fb"}
    bias_sb = {d: sb(f"bias_sb{d}", [1, NSL], BF16).ap() for d in "fb"}
    ones_sb = sb("ones_sb", [1, 128], BF16).ap()
    ident = sb("ident", [64, 64], F32).ap()
    rcv = {d: [sb(f"rcv{d}{p}", [128, KH * B], BF16).ap() for p in range(2)]
           for d in "fb"}
    snd = {d: [sb(f"snd{d}{p}", [128, B], BF16).ap() for p in range(2)]
           for d in "fb"}
    xgb = {d: [sb(f"xgb{d}{m}", [B, NSL], BF16).ap() for m in range(3)]
           for d in "fb"}
    gadd = {d: sb(f"gadd{d}", [B, NSL], F32).ap() for d in "fb"}
    acts = {d: sb(f"acts{d}", [B, NSL], F32).ap() for d in "fb"}
    c_sb = {d: sb(f"c{d}", [B, HSL], F32).ap() for d in "fb"}
    tnc = {d: sb(f"tnc{d}", [B, HSL], F32).ap() for d in "fb"}
    t1_sb = {d: sb(f"t1{d}", [B, HSL], F32).ap() for d in "fb"}
    t2_sb = {d: sb(f"t2{d}", [B, HSL], F32).ap() for d in "fb"}
    hbuf = {d: [sb(f"hb{d}{p}", [B, HSL], F32).ap() for p in range(2)]
            for d in "fb"}
    xT = [sb(f"xT{m}", [128, KI * 128], BF16).ap() for m in range(4)]
    ot = [sb(f"ot{m}", [128, NSL], BF16).ap() for m in range(2)]

    # ---- PSUM static ----
    ap_ = nc.alloc_psum_tensor
    ps1 = [ap_(f"ps1{m}", [128, NSL], F32).ap() for m in range(2)]
    g_ps = {d: ap_(f"gps{d}", [B, NSL], F32).ap() for d in "fb"}
    tps = {d: [ap_(f"tps{d}{p}", [128, B], F32).ap() for p in range(2)]
           for d in "fb"}

    # ---- prologue ----
    for d in "fb":
        nc.sync.dma_start(
            whhT_sb[d].rearrange("p (k n) -> p k n", n=NSL),
            whhT_d[d].ap().rearrange("(k p) n -> p k n", p=128),
        ).then_inc(sem["sw"], 16)
        nc.sync.dma_start(
            wihT_sb[d].rearrange("p (k n) -> p k n", n=NSL),
            wihT_d[d].ap().rearrange("(k p) n -> p k n", p=128),
        ).then_inc(sem["sw"], 16)
        nc.sync.dma_start(bias_sb[d], bias_d[d].ap()).then_inc(sem["sw"], 16)

    nc.vector.memset(ones_sb, 1.0).then_inc(sem["initv"], 1)
    for d in "fb":
        nc.vector.memset(rcv[d][0], 0.0).then_inc(sem["initv"], 1)
        nc.vector.memset(c_sb[d], 0.0).then_inc(sem["initv"], 1)
    # identity for PE transpose (f32)
    nc.gpsimd.memset(ident, 0.0)
    nc.gpsimd.affine_select(
        out=ident, in_=ident, compare_op=OP.not_equal, fill=1.0,
        base=0, pattern=[[-1, 64]], channel_multiplier=1,
    ).then_inc(sem["initg"], 1)
    pid = nc.gpsimd.partition_id()

    # PE waits once for all the setup
    nc.tensor.wait_ge(sem["sw"], 16 * 6)
    nc.tensor.wait_ge(sem["initv"], 5)
    nc.tensor.wait_ge(sem["initg"], 1)

    # ---- phase 1: xg[d] = x[d] @ wihT[d] + bias[d]  (time-major out) ----
    cidx = 0
    for d in "fb":
        xg3 = xg_d[d].ap().rearrange("(s b) n -> s b n", b=B)
        for b in range(B):
            for sq in range(TCH):
                m2 = cidx % 2
                m4 = cidx % 4
                sxT = sem[f"sxT{m4}"]
                use = cidx // 4 + 1
                # in-DMAs (transpose): x[b, s-slice, k-chunk] -> xT[m4][:, k]
                if cidx >= 4:
                    nc.sync.wait_ge(sem["mm1"], cidx - 3)
                for k in range(KI):
                    nc.sync.dma_start(
                        xT[m4][:, ts(k, 128)],
                        x_d[d].ap()[b, ds(128 * sq, 128), ts(k, 128)],
                        transpose=True,
                    ).then_inc(sxT, 16)
                # matmuls
                nc.tensor.wait_ge(sxT, 128 * use)
                if cidx >= 2:
                    nc.tensor.wait_ge(sem["evac1"], cidx - 1)
                for k in range(KI):
                    nc.tensor.matmul(ps1[m2], xT[m4][:, ts(k, 128)],
                                     wihT_sb[d][:, ts(k, NSL)],
                                     start=(k == 0), stop=False)
                nc.tensor.matmul(ps1[m2], ones_sb, bias_sb[d],
                                 start=False, stop=True).then_inc(sem["mm1"], 1)
                # evac
                nc.vector.wait_ge(sem["mm1"], cidx + 1)
                nc.vector.tensor_copy(ot[m2], ps1[m2]).then_inc(sem["evac1"], 1)
                # out
                nc.sync.wait_ge(sem["evac1"], cidx + 1)
                nc.sync.dma_start(xg3[ds(128 * sq, 128), b, :],
                                  ot[m2]).then_inc(sem["p1out"], 16)
                cidx += 1

    # ---- phase 2 ----
    RD = [(0, k) for k in range(NCORES)]
    # xg prefetch for steps 0..2 (after all phase-1 writes land)
    nc.sync.wait_ge(sem["p1out"], 16 * cidx)
    for d in "fb":
        for u in range(min(3, S)):
            nc.sync.dma_start(xgb[d][u], xg_d[d].ap()[ds(B * u, B), :]
                              ).then_inc(sem[f"sxg_{d}{u}"], 16)

    ho2 = {d: hout_d[d].ap().rearrange("b s h -> b (s h)") for d in "fb"}

    for t in range(S):
        p = t % 2
        m3 = t % 3
        # ---------- SP: xg prefetch t+3, hout t ----------
        for d in "fb":
            if t + 3 < S:
                nc.sync.wait_ge(sem[f"add_{d}"], t + 1)
                nc.sync.dma_start(xgb[d][m3],
                                  xg_d[d].ap()[ds(B * (t + 3), B), :]
                                  ).then_inc(sem[f"sxg_{d}{m3}"], 16)
        # ---------- PE: matmuls ----------
        for d in "fb":
            if t >= 1:
                # sim: single-core TimelineSim drops cross-core RemoteSemUpdate;
                # wait on the local-completion sem of the same broadcast instead.
                if sim:
                    nc.tensor.wait_ge(sem[f"l_{d}{1 - p}"], 16 * ((t + 1) // 2))
                else:
                    nc.tensor.wait_ge(sem[f"r_{d}{p}"], 16 * ((t + 1) // 2))
                nc.tensor.wait_ge(sem[f"add_{d}"], t)
            for k in range(KH):
                mm = nc.tensor.matmul(g_ps[d], rcv[d][p][:, ts(k, B)],
                                      whhT_sb[d][:, ts(k, NSL)],
                                      start=(k == 0), stop=(k == KH - 1))
            mm.then_inc(sem[f"mm_{d}"], 1)
        # ---------- DVE: gate add ----------
        for d in "fb":
            nc.vector.wait_ge(sem[f"mm_{d}"], t + 1)
            nc.vector.wait_ge(sem[f"sxg_{d}{m3}"], 16 * (t // 3 + 1))
            nc.vector.tensor_tensor(gadd[d], g_ps[d], xgb[d][m3],
                                    op=OP.add).then_inc(sem[f"add_{d}"], 1)
        # ---------- ACT: activations ----------
        for d in "fb":
            nc.scalar.wait_ge(sem[f"add_{d}"], t + 1)
            nc.scalar.activation(acts[d][:, ds(0, 384)], gadd[d][:, ds(0, 384)],
                                 AF.Sigmoid)
            nc.scalar.activation(acts[d][:, ds(384, 128)],
                                 gadd[d][:, ds(384, 128)],
                                 AF.Tanh).then_inc(sem[f"act_{d}"], 1)
        # ---------- DVE: c update ----------
        for d in "fb":
            nc.vector.wait_ge(sem[f"act_{d}"], t + 1)
            nc.vector.tensor_tensor(t1_sb[d], acts[d][:, ds(128, 128)],
                                    c_sb[d], op=OP.mult)
            nc.vector.tensor_tensor(t2_sb[d], acts[d][:, ds(0, 128)],
                                    acts[d][:, ds(384, 128)], op=OP.mult)
            nc.vector.tensor_tensor(c_sb[d], t1_sb[d], t2_sb[d],
                                    op=OP.add).then_inc(sem[f"c_{d}"], 1)
        # ---------- ACT: tanh(c) ----------
        for d in "fb":
            nc.scalar.wait_ge(sem[f"c_{d}"], t + 1)
            nc.scalar.activation(tnc[d], c_sb[d],
                                 AF.Tanh).then_inc(sem[f"tc_{d}"], 1)
        # ---------- DVE: h ----------
        for d in "fb":
            nc.vector.wait_ge(sem[f"tc_{d}"], t + 1)
            if t >= 2:
                nc.vector.wait_ge(sem[f"shd_{d}{p}"], 16 * (t // 2))
            nc.vector.tensor_tensor(hbuf[d][p], acts[d][:, ds(256, 128)],
                                    tnc[d], op=OP.mult
                                    ).then_inc(sem[f"h_{d}"], 1)
        # ---------- SP: hout ----------
        for d in "fb":
            nc.sync.wait_ge(sem[f"h_{d}"], t + 1)
            nc.sync.dma_start(ho2[d][:, ds(t * HSL, HSL)], hbuf[d][p]
                              ).then_inc(sem[f"shd_{d}{p}"], 16)
        # ---------- PE: transpose h ----------
        for d in "fb":
            nc.tensor.wait_ge(sem[f"h_{d}"], t + 1)
            if t >= 2:
                nc.tensor.wait_ge(sem[f"cast_{d}"], t - 1)
            nc.tensor.transpose(tps[d][p], hbuf[d][p],
                                ident).then_inc(sem[f"T_{d}"], 1)
        # ---------- ACT: cast h^T -> bf16 snd (keeps DVE off the path) ----------
        for d in "fb":
            nc.scalar.wait_ge(sem[f"T_{d}"], t + 1)
            if t >= 2:
                nc.scalar.wait_ge(sem[f"l_{d}{p}"], 16 * (t // 2))
            nc.scalar.activation(snd[d][p], tps[d][p],
                                 AF.Copy).then_inc(sem[f"cast_{d}"], 1)
        # ---------- POOL: broadcast ----------
        for d in "fb":
            nc.gpsimd.remote_dma_broadcast(
                rcv[d][(t + 1) % 2][:, ds(pid * B, B)], snd[d][p],
                remote_sem=sem[f"r_{d}{(t + 1) % 2}"],
                local_sem=sem[f"l_{d}{p}"],
                rdests=RD).then_inc(sem[f"prep_{d}"], 1)
        for d in "fb":
            nc.gpsimd.wait_ge(sem[f"prep_{d}"], t + 1)
            nc.gpsimd.wait_ge(sem[f"cast_{d}"], t + 1)
            nc.gpsimd.trigger_dma(count=1)

    # ---- epilogue: drain all async traffic before NEFF end ----
    assert S % 2 == 0
    for d in "fb":
        for p in range(2):
            nc.sync.wait_ge(sem[f"shd_{d}{p}"], 16 * (S // 2))
            nc.sync.wait_ge(sem[f"l_{d}{p}"], 16 * (S // 2))
            if not sim:
                nc.sync.wait_ge(sem[f"r_{d}{p}"], 16 * (S // 2))

    nc.compile()
    nc.has_collectives = True  # force PJRT co-scheduling
    return nc


_CACHE = {}


def _get(S):
    if S not in _CACHE:
        _CACHE[S] = build(S)
    return _CACHE[S]


def _host_shard(inputs, S):
    fx = np.asarray(inputs["forward_x"], np.float32)[:, :S]
    bx = np.asarray(inputs["backward_x"], np.float32)[:, :S]
    xf = np.ascontiguousarray(fx).astype(BF16_NP)
    xb = np.ascontiguousarray(bx[:, ::-1]).astype(BF16_NP)
    maps = []
    for r in range(NCORES):
        rows = np.concatenate([
            np.arange(128 * r, 128 * r + 128),             # i
            np.arange(H + 128 * r, H + 128 * r + 128),     # f
            np.arange(3 * H + 128 * r, 3 * H + 128 * r + 128),  # o
            np.arange(2 * H + 128 * r, 2 * H + 128 * r + 128),  # g~
        ])
        m = {"xf": xf, "xb": xb}
        for d, sfx in (("f", "_f"), ("b", "_b")):
            wih = np.asarray(inputs[f"W_ih{sfx}"], np.float32)[rows]
            whh = np.asarray(inputs[f"W_hh{sfx}"], np.float32)[rows]
            bias = (np.asarray(inputs[f"b_ih{sfx}"], np.float32)
                    + np.asarray(inputs[f"b_hh{sfx}"], np.float32))[rows]
            m[f"wihT{d}"] = np.ascontiguousarray(wih.T).astype(BF16_NP)
            m[f"whhT{d}"] = np.ascontiguousarray(whh.T).astype(BF16_NP)
            m[f"bias{d}"] = bias.reshape(1, -1).astype(BF16_NP)
        maps.append(m)
    return maps


def run(inputs, S=S_FULL, trace=False, **_):
    maps = _host_shard(inputs, S)
    nc = _get(S)
    t0 = time.time()
    res = run_bass_kernel_spmd(nc, maps, core_ids=list(range(NCORES)),
                               trace=trace)
    wall = time.time() - t0
    outs = res.results
    fwd = np.concatenate([outs[r]["hf"] for r in range(NCORES)], axis=2)
    bwd = np.concatenate([outs[r]["hb"] for r in range(NCORES)], axis=2)[:, ::-1]
    return (fwd, bwd), res, wall


def kernel(**inputs):
    (fwd, bwd), _, _ = run(inputs)
    return fwd.astype(np.float32), bwd.astype(np.float32)


def run_timed(inputs, S=S_FULL, iters=3):
    """Mirror bass2jax.run_bass_via_pjrt but pre-stage device inputs and time
    pure execution (incl. PJRT dispatch, excl. H2D of the big tensors)."""
    import jax
    import jax.numpy as jnp
    from jax.sharding import Mesh, PartitionSpec
    from jax.experimental.shard_map import shard_map
    import concourse.mybir as mb
    from concourse.bass2jax import (_bass_exec_p, partition_id_tensor,
                                    install_neuronx_cc_hook)

    maps = _host_shard(inputs, S)
    nc = _get(S)
    install_neuronx_cc_hook()

    partition_name = nc.partition_id_tensor.name if nc.partition_id_tensor else None
    in_names, out_names, out_avals, zero_outs = [], [], [], []
    for alloc in nc.m.functions[0].allocations:
        if not isinstance(alloc, mb.MemoryLocationSet):
            continue
        name = alloc.memorylocations[0].name
        if alloc.kind == "ExternalInput":
            if name != partition_name:
                in_names.append(name)
        elif alloc.kind == "ExternalOutput":
            out_names.append(name)
            shape = tuple(alloc.tensor_shape)
            dtype = mb.dt.np(alloc.dtype)
            out_avals.append(jax.core.ShapedArray(shape, dtype))
            zero_outs.append(np.zeros(shape, dtype))
    n_params = len(in_names)
    n_outs = len(out_avals)
    all_in_names = list(in_names) + out_names
    if partition_name is not None:
        all_in_names.append(partition_name)
    donate = tuple(range(n_params, n_params + n_outs))

    def _body(*args):
        operands = list(args)
        if partition_name is not None:
            operands.append(partition_id_tensor())
        return tuple(_bass_exec_p.bind(
            *operands, out_avals=tuple(out_avals), in_names=tuple(all_in_names),
            out_names=tuple(out_names), lowering_input_output_aliases=(),
            sim_require_finite=True, sim_require_nnan=True, nc=nc))

    devices = jax.devices()[:NCORES]
    mesh = Mesh(np.asarray(devices), ("core",))
    in_specs = (PartitionSpec("core"),) * (n_params + n_outs)
    out_specs = (PartitionSpec("core"),) * n_outs
    sharded = jax.jit(shard_map(_body, mesh=mesh, in_specs=in_specs,
                                out_specs=out_specs, check_rep=False),
                      donate_argnums=donate, keep_unused=True)
    sharding = jax.sharding.NamedSharding(mesh, PartitionSpec("core"))
    concat_in = [
        jax.device_put(
            np.concatenate([np.asarray(maps[c][nm]) for c in range(NCORES)],
                           axis=0), sharding)
        for nm in in_names]
    jax.block_until_ready(concat_in)

    times = []
    out_arrs = None
    for it in range(iters):
        zeros = [jax.device_put(
            np.zeros((NCORES * z.shape[0], *z.shape[1:]), z.dtype), sharding)
            for z in zero_outs]
        jax.block_until_ready(zeros)
        t0 = time.time()
        out_arrs = sharded(*concat_in, *zeros)
        jax.block_until_ready(out_arrs)
        times.append(time.time() - t0)
    res = {name: np.asarray(out_arrs[i]).reshape(NCORES, *out_avals[i].shape)
           for i, name in enumerate(out_names)}
    fwd = np.concatenate([res["hf"][r] for r in range(NCORES)], axis=2)
    bwd = np.concatenate([res["hb"][r] for r in range(NCORES)], axis=2)[:, ::-1]
    return (fwd, bwd), times



# revision 6
# speedup vs baseline: 3.8592x; 3.8592x over previous
"""BiLSTM on 8 TRN2 cores — v2.

Sharding: gate-split. Core r owns H-dims [128r, 128r+128) of both directions
(host reorders gate rows to [i|f|o|g~]). Both directions are MERGED into
single 128-partition tiles: partitions 0-63 = forward batch, 64-127 =
backward batch. One combined h^T broadcast per step ([128, 128] bf16),
descriptor prep issued 3 steps ahead of its trigger.

Phase 1: xg = x @ W_ih^T + bias via one [128s x 1024i] DMA-transpose per
(dir, batch, s-quarter) tile-set; bias added during the PSUM->SBUF evac on
DVE. xg stored time-major in DRAM as [S, 128, 512].

dist=True: per-core x input is only its 8 batch rows; each core transposes
its own rows and broadcasts the xT tile-sets to all cores, which consume
them directly from SBUF (flow control via credit sem broadcasts).

Phase 2: per step: PE accumulates xg into PSUM via an identity-stationary
matmul, then 16 matmuls (8 k-chunks x 2 directions col-tiled); ACT does
sigmoid/tanh straight from PSUM; DVE updates c and h (h in bf16); PE
transposes h; ACT casts to the send buffer; POOL triggers the pre-prepped
combined broadcast.
"""

import sys
import time

import numpy as np
import ml_dtypes

sys.path.insert(0, "/opt/trn_rl_repo")

import concourse.bass as bass
import concourse.mybir as mybir
from concourse import bacc
from concourse.bass import ds, ts
from concourse.bass_utils import run_bass_kernel_spmd

F32 = mybir.dt.float32
BF16 = mybir.dt.bfloat16
AF = mybir.ActivationFunctionType
OP = mybir.AluOpType
BF16_NP = ml_dtypes.bfloat16

B, S_FULL, I_IN, H = 64, 512, 1024, 1024
NSL = 512            # gate dims per core ([i|f|o|g~] x 128)
HSL = 128            # h dims per core
NCORES = 8
BLOC = B // NCORES   # 8 batch rows per core in dist mode

# dma_start_transpose of [128, 1024] -> [128, 8, 128]: imap[p, k] = source
# column landing at out[p, k, :]. Set by tprobe: True -> i = k*128 + p
# (natural chunking), False -> i = p*8 + k (interleaved).
TRANSPOSE_NATURAL = True


def _iperm():
    """Permutation: wihT row j (SBUF chunk k, partition p) = W_ih col imap."""
    p = np.arange(128)
    k = np.arange(8)
    if TRANSPOSE_NATURAL:
        imap = k[:, None] * 128 + p[None, :]   # [k, p]
    else:
        imap = p[None, :] * 8 + k[:, None]
    return imap.reshape(-1)                    # flat [1024]: (k, p) order


def build(S=S_FULL, dist=True):
    KI = I_IN // 128   # 8
    KH = H // 128      # 8
    TCH = S // 128     # s-quarters per batch row
    assert S % 128 == 0 and S % 2 == 0

    nc = bacc.Bacc("TRN2", target_bir_lowering=False, debug=False,
                   num_devices=NCORES)

    # ---- DRAM ----
    nb = BLOC if dist else B
    x_d = {d: nc.dram_tensor(f"x{d}", [nb, S, I_IN], BF16,
                             kind="ExternalInput") for d in "fb"}
    wihT_d = {d: nc.dram_tensor(f"wihT{d}", [I_IN, NSL], BF16,
                                kind="ExternalInput") for d in "fb"}
    whhT_d = {d: nc.dram_tensor(f"whhT{d}", [H, NSL], BF16,
                                kind="ExternalInput") for d in "fb"}
    bias_d = {d: nc.dram_tensor(f"bias{d}", [128, NSL], BF16,
                                kind="ExternalInput") for d in "fb"}
    hout_d = nc.dram_tensor("hc", [128, S, HSL], BF16, kind="ExternalOutput")
    xg_d = nc.dram_tensor("xgc", [S, 128, NSL], BF16, kind="Internal")

    # ---- semaphores ----
    sem = {}
    def SEM(name):
        sem[name] = nc.alloc_semaphore(name)
        return sem[name]
    for nm in ("mm", "act", "cs", "tc", "h", "T", "cast", "mm1", "ev1",
               "p1out", "sw", "initv", "initg", "shd0", "shd1",
               "r0", "r1", "l0", "l1", "sxg0", "sxg1", "sxg2",
               "sxT0", "sxT1", "sxT2", "sxT3",
               "rx", "lx", "cred", "credl", "mmx"):
        SEM(nm)

    # ---- SBUF ----
    sb = nc.alloc_sbuf_tensor
    whhT_sb = {d: sb(f"whhT_sb{d}", [128, KH, NSL], BF16).ap() for d in "fb"}
    wihT_sb = {d: sb(f"wihT_sb{d}", [128, KI, NSL], BF16).ap() for d in "fb"}
    bias_sb = {d: sb(f"bias_sb{d}", [128, NSL], BF16).ap() for d in "fb"}
    id_bf = sb("id_bf", [128, 128], BF16).ap()
    id_f32 = sb("id_f32", [128, 128], F32).ap()
    rcv = [sb(f"rcv{p}", [128, KH * 128], BF16).ap() for p in range(2)]
    snd = [sb(f"snd{p}", [128, 128], BF16).ap() for p in range(2)]
    xgb = [sb(f"xgb{m}", [128, NSL], BF16).ap() for m in range(3)]
    acts = sb("acts", [128, NSL], F32).ap()
    c_sb = sb("c", [128, HSL], F32).ap()
    tnc = sb("tnc", [128, HSL], F32).ap()
    t1_sb = sb("t1", [128, HSL], F32).ap()
    t2_sb = sb("t2", [128, HSL], F32).ap()
    hbuf = [sb(f"hb{p}", [128, HSL], BF16).ap() for p in range(2)]
    ot = [sb(f"ot{m}", [128, NSL], BF16).ap() for m in range(2)]
    if dist:
        # 2-slot ring of incoming xT tile-sets, 8 senders each
        rcvx = [sb(f"rcvx{q}", [128, NCORES, KI, 128], BF16).ap()
                for q in range(2)]
        xTs = [sb(f"xTs{m}", [128, KI, 128], BF16).ap() for m in range(2)]
    else:
        xT = [sb(f"xT{m}", [128, KI, 128], BF16).ap() for m in range(4)]

    # ---- PSUM ----
    ap_ = nc.alloc_psum_tensor
    ps1 = [ap_(f"ps1{m}", [128, NSL], F32).ap() for m in range(2)]
    g_ps = [ap_(f"gps{p}", [128, NSL], F32).ap() for p in range(2)]
    tps = [ap_(f"tps{p}", [128, 128], BF16).ap() for p in range(2)]

    # ---- prologue ----
    for d in "fb":
        nc.sync.dma_start(
            whhT_sb[d], whhT_d[d].ap().rearrange("(k p) n -> p k n", p=128),
        ).then_inc(sem["sw"], 16)
        nc.sync.dma_start(
            wihT_sb[d], wihT_d[d].ap().rearrange("(k p) n -> p k n", p=128),
        ).then_inc(sem["sw"], 16)
        nc.sync.dma_start(bias_sb[d], bias_d[d].ap()).then_inc(sem["sw"], 16)

    nc.vector.memset(rcv[0], 0.0).then_inc(sem["initv"], 1)
    nc.vector.memset(c_sb, 0.0).then_inc(sem["initv"], 1)
    NINITV = 2
    for idt in (id_bf, id_f32):
        nc.gpsimd.memset(idt, 0.0)
        nc.gpsimd.affine_select(
            out=idt, in_=idt, compare_op=OP.not_equal, fill=1.0,
            base=0, pattern=[[-1, 128]], channel_multiplier=1,
        ).then_inc(sem["initg"], 1)
    pid = nc.gpsimd.partition_id()

    nc.tensor.wait_ge(sem["sw"], 16 * 6)
    nc.tensor.wait_ge(sem["initv"], NINITV)
    nc.tensor.wait_ge(sem["initg"], 2)

    RD = [(0, k) for k in range(NCORES)]

    # =====================  phase 1  =====================
    xg4 = xg_d.ap().rearrange("(sq s) r n -> sq s r n", s=128)

    def p1_consume(src_ap, d, bg, sq, cidx):
        """8 matmuls + bias-evac + xg write for one tile-set.
        src_ap: [128, KI, 128] xT tile-set (i-chunks on partitions)."""
        m2 = cidx % 2
        if cidx >= 2:
            nc.tensor.wait_ge(sem["ev1"], cidx - 1)
        for k in range(KI):
            mm = nc.tensor.matmul(ps1[m2], src_ap[:, k, :],
                                  wihT_sb[d][:, k, :],
                                  start=(k == 0), stop=(k == KI - 1))
        mm.then_inc(sem["mm1"], 1)
        nc.vector.wait_ge(sem["mm1"], cidx + 1)
        nc.vector.tensor_tensor(ot[m2], ps1[m2], bias_sb[d],
                                op=OP.add).then_inc(sem["ev1"], 1)
        nc.sync.wait_ge(sem["ev1"], cidx + 1)
        row = bg if d == "f" else B + bg
        nc.sync.dma_start(xg4[sq, :, row, :], ot[m2]).then_inc(sem["p1out"], 16)

    if not dist:
        cidx = 0
        for d in "fb":
            for bg in range(B):
                for sq in range(TCH):
                    m4 = cidx % 4
                    sxT = sem[f"sxT{m4}"]
                    if cidx >= 4:
                        nc.sync.wait_ge(sem["mm1"], cidx - 3)
                    nc.sync.dma_start(xT[m4], x_d[d].ap()[bg, ds(128 * sq, 128), :],
                                      transpose=True).then_inc(sxT, 16)
                    nc.tensor.wait_ge(sxT, 16 * (cidx // 4 + 1))
                    p1_consume(xT[m4], d, bg, sq, cidx)
                    cidx += 1
    else:
        # round u = (d, b_local, sq); every core broadcasts its tile-set for
        # round u, then consumes all 8 senders' tile-sets. Credits for round
        # u-1 are sent during round u (lag-1) so bcast(u) overlaps MMs(u).
        NR = 2 * BLOC * TCH
        cidx = 0

        def send_credit():
            nc.gpsimd.remote_sem_update_broadcast(
                remote_sem=sem["cred"], local_sem=sem["credl"], rdests=RD)
            nc.gpsimd.trigger_dma(count=1)

        for u in range(NR):
            d = "f" if u < NR // 2 else "b"
            ul = u % (NR // 2)
            bl, sq = ul // TCH, ul % TCH
            m2u = u % 2
            # SP: transpose own tile-set into xTs[m2u]
            if u >= 2:
                nc.sync.wait_ge(sem["lx"], 16 * (u - 1))
            nc.sync.dma_start(xTs[m2u], x_d[d].ap()[bl, ds(128 * sq, 128), :],
                              transpose=True).then_inc(sem[f"sxT{m2u}"], 16)
            # POOL: credit round u-1 (its MMs are done), then fire bcast(u)
            if u >= 1:
                nc.gpsimd.wait_ge(sem["mm1"], NCORES * u)
                send_credit()
            nc.gpsimd.remote_dma_broadcast(
                rcvx[m2u].rearrange("p r k s -> p (r k s)")[
                    :, ds(pid * KI * 128, KI * 128)],
                xTs[m2u].rearrange("p k s -> p (k s)"),
                remote_sem=sem["rx"], local_sem=sem["lx"], rdests=RD)
            nc.gpsimd.wait_ge(sem[f"sxT{m2u}"], 16 * (u // 2 + 1))
            if u >= 2:
                # all receivers consumed slot m2u (round u-2): cred counts
                # rounds 0..u-2 = u-1 credit broadcasts of 16 total each
                nc.gpsimd.wait_ge(sem["cred"], 16 * (u - 1))
            nc.gpsimd.trigger_dma(count=1)
            # PE+DVE+SP: consume all 8 senders' tile-sets of round u
            nc.tensor.wait_ge(sem["rx"], 16 * (u + 1))
            for k_s in range(NCORES):
                p1_consume(rcvx[m2u][:, k_s, :, :], d, k_s * BLOC + bl, sq,
                           cidx)
                cidx += 1
        # final credit (round NR-1) so peers' epilogue drains
        nc.gpsimd.wait_ge(sem["mm1"], NCORES * NR)
        send_credit()

    NT = cidx  # 2*B*TCH tiles total

    # =====================  phase 2  =====================
    nc.sync.wait_ge(sem["p1out"], 16 * NT)
    for u in range(min(3, S)):
        nc.sync.dma_start(xgb[u], xg_d.ap()[u, :, :]).then_inc(sem[f"sxg{u}"], 16)

    def issue_prep(t):
        p = t % 2
        nc.gpsimd.remote_dma_broadcast(
            rcv[(t + 1) % 2][:, ds(pid * 128, 128)], snd[p],
            remote_sem=sem[f"r{(t + 1) % 2}"], local_sem=sem[f"l{p}"],
            rdests=RD)

    for u in range(min(3, S)):
        issue_prep(u)

    ho2 = hout_d.ap()

    for t in range(S):
        p = t % 2
        m3 = t % 3
        # ---- SP ----
        if t + 3 < S:
            nc.sync.wait_ge(sem["mm"], t + 1)
            nc.sync.dma_start(xgb[m3], xg_d.ap()[t + 3, :, :]
                              ).then_inc(sem[f"sxg{m3}"], 16)
        nc.sync.wait_ge(sem["h"], t + 1)
        nc.sync.dma_start(ho2[:, t, :], hbuf[p]).then_inc(sem[f"shd{p}"], 16)
        # ---- PE: I-add then recurrent matmuls ----
        if t >= 2:
            nc.tensor.wait_ge(sem["act"], t - 1)     # g_ps[p] free
        nc.tensor.wait_ge(sem[f"sxg{m3}"], 16 * (t // 3 + 1))
        nc.tensor.matmul(g_ps[p], id_bf, xgb[m3], start=True, stop=False)
        if t >= 1:
            nc.tensor.wait_ge(sem[f"r{p}"], 16 * ((t + 1) // 2))
        for k in range(KH):
            last = k == KH - 1
            nc.tensor.matmul(g_ps[p][0:64, :], rcv[p][:, ds(k * 128, 64)],
                             whhT_sb["f"][:, k, :], start=False, stop=False)
            mmb = nc.tensor.matmul(g_ps[p][64:128, :],
                                   rcv[p][:, ds(k * 128 + 64, 64)],
                                   whhT_sb["b"][:, k, :],
                                   start=False, stop=last)
        mmb.then_inc(sem["mm"], 1)
        # ---- ACT: activations from PSUM ----
        nc.scalar.wait_ge(sem["mm"], t + 1)
        nc.scalar.activation(acts[:, ds(0, 384)], g_ps[p][:, ds(0, 384)],
                             AF.Sigmoid)
        nc.scalar.activation(acts[:, ds(384, 128)], g_ps[p][:, ds(384, 128)],
                             AF.Tanh).then_inc(sem["act"], 1)
        # ---- DVE: c update ----
        nc.vector.wait_ge(sem["act"], t + 1)
        nc.vector.tensor_tensor(t2_sb, acts[:, ds(0, 128)],
                                acts[:, ds(384, 128)], op=OP.mult)
        nc.vector.tensor_tensor(t1_sb, acts[:, ds(128, 128)], c_sb,
                                op=OP.mult)
        nc.vector.tensor_tensor(c_sb, t1_sb, t2_sb,
                                op=OP.add).then_inc(sem["cs"], 1)
        # ---- ACT: tanh(c) ----
        nc.scalar.wait_ge(sem["cs"], t + 1)
        nc.scalar.activation(tnc, c_sb, AF.Tanh).then_inc(sem["tc"], 1)
        # ---- DVE: h (bf16) ----
        nc.vector.wait_ge(sem["tc"], t + 1)
        if t >= 2:
            nc.vector.wait_ge(sem[f"shd{p}"], 16 * (t // 2))
        nc.vector.tensor_tensor(hbuf[p], acts[:, ds(256, 128)], tnc,
                                op=OP.mult).then_inc(sem["h"], 1)
        # ---- PE: transpose h ----
        nc.tensor.wait_ge(sem["h"], t + 1)
        if t >= 2:
            nc.tensor.wait_ge(sem["cast"], t - 1)    # tps[p] free
        nc.tensor.transpose(tps[p], hbuf[p], id_bf).then_inc(sem["T"], 1)
        # ---- ACT: copy h^T -> snd ----
        nc.scalar.wait_ge(sem["T"], t + 1)
        if t >= 2:
            nc.scalar.wait_ge(sem[f"l{p}"], 16 * (t // 2))
        nc.scalar.activation(snd[p], tps[p], AF.Copy).then_inc(sem["cast"], 1)
        # ---- POOL: fire the pre-prepped broadcast ----
        nc.gpsimd.wait_ge(sem["cast"], t + 1)
        nc.gpsimd.trigger_dma(count=1)
        if t + 3 < S:
            issue_prep(t + 3)

    # ---- epilogue ----
    for p in range(2):
        nc.sync.wait_ge(sem[f"shd{p}"], 16 * (S // 2))
        nc.sync.wait_ge(sem[f"l{p}"], 16 * (S // 2))
        nc.sync.wait_ge(sem[f"r{p}"], 16 * (S // 2))
    if dist:
        NR = 2 * BLOC * TCH
        nc.sync.wait_ge(sem["lx"], 16 * NR)
        nc.sync.wait_ge(sem["rx"], 16 * NR)
        nc.sync.wait_ge(sem["cred"], 16 * NR)
        nc.sync.wait_ge(sem["credl"], 16 * NR)

    nc.compile()
    nc.has_collectives = True
    return nc


_CACHE = {}


def _get(S, dist=True):
    key = (S, dist)
    if key not in _CACHE:
        _CACHE[key] = build(S, dist)
    return _CACHE[key]


def _host_shard(inputs, S, dist=True):
    fx = np.asarray(inputs["forward_x"], np.float32)[:, :S]
    bx = np.asarray(inputs["backward_x"], np.float32)[:, :S]
    xf = np.ascontiguousarray(fx).astype(BF16_NP)
    xb = np.ascontiguousarray(bx[:, ::-1]).astype(BF16_NP)
    iperm = _iperm()
    maps = []
    for r in range(NCORES):
        rows = np.concatenate([
            np.arange(128 * r, 128 * r + 128),                  # i
            np.arange(H + 128 * r, H + 128 * r + 128),          # f
            np.arange(3 * H + 128 * r, 3 * H + 128 * r + 128),  # o
            np.arange(2 * H + 128 * r, 2 * H + 128 * r + 128),  # g~
        ])
        if dist:
            m = {"xf": xf[BLOC * r:BLOC * (r + 1)],
                 "xb": xb[BLOC * r:BLOC * (r + 1)]}
        else:
            m = {"xf": xf, "xb": xb}
        for d, sfx in (("f", "_f"), ("b", "_b")):
            wih = np.asarray(inputs[f"W_ih{sfx}"], np.float32)[rows]
            whh = np.asarray(inputs[f"W_hh{sfx}"], np.float32)[rows]
            bias = (np.asarray(inputs[f"b_ih{sfx}"], np.float32)
                    + np.asarray(inputs[f"b_hh{sfx}"], np.float32))[rows]
            # wihT rows permuted to match the transpose tile layout
            m[f"wihT{d}"] = np.ascontiguousarray(
                wih.T[iperm]).astype(BF16_NP)
            m[f"whhT{d}"] = np.ascontiguousarray(whh.T).astype(BF16_NP)
            m[f"bias{d}"] = np.broadcast_to(
                bias.astype(BF16_NP), (128, NSL)).copy()
        maps.append(m)
    return maps


_EXEC_CACHE = {}


def _get_exec(S, dist):
    """Cached jitted executable + metadata for the (S, dist) program."""
    key = (S, dist)
    if key in _EXEC_CACHE:
        return _EXEC_CACHE[key]
    import jax
    import jax.numpy as jnp
    from jax.sharding import Mesh, PartitionSpec
    from jax.experimental.shard_map import shard_map
    import concourse.mybir as mb
    from concourse.bass2jax import (_bass_exec_p, partition_id_tensor,
                                    install_neuronx_cc_hook)
    install_neuronx_cc_hook()
    nc = _get(S, dist)
    partition_name = (nc.partition_id_tensor.name
                      if nc.partition_id_tensor else None)
    in_names, out_names, out_avals = [], [], []
    for alloc in nc.m.functions[0].allocations:
        if not isinstance(alloc, mb.MemoryLocationSet):
            continue
        name = alloc.memorylocations[0].name
        if alloc.kind == "ExternalInput":
            if name != partition_name:
                in_names.append(name)
        elif alloc.kind == "ExternalOutput":
            out_names.append(name)
            out_avals.append(jax.core.ShapedArray(tuple(alloc.tensor_shape),
                                                  mb.dt.np(alloc.dtype)))
    n_params = len(in_names)
    n_outs = len(out_avals)
    all_in = list(in_names) + out_names
    if partition_name is not None:
        all_in.append(partition_name)

    def _body(*args):
        operands = list(args)
        if partition_name is not None:
            operands.append(partition_id_tensor())
        return tuple(_bass_exec_p.bind(
            *operands, out_avals=tuple(out_avals), in_names=tuple(all_in),
            out_names=tuple(out_names), lowering_input_output_aliases=(),
            sim_require_finite=True, sim_require_nnan=True, nc=nc))

    devices = jax.devices()[:NCORES]
    mesh = Mesh(np.asarray(devices), ("core",))
    sharded = jax.jit(
        shard_map(_body, mesh=mesh,
                  in_specs=(PartitionSpec("core"),) * (n_params + n_outs),
                  out_specs=(PartitionSpec("core"),) * n_outs,
                  check_rep=False),
        donate_argnums=tuple(range(n_params, n_params + n_outs)),
        keep_unused=True)
    sharding = jax.sharding.NamedSharding(mesh, PartitionSpec("core"))

    @jax.jit
    def _mkzeros():
        return tuple(
            jnp.zeros((NCORES * a.shape[0], *a.shape[1:]),
                      a.dtype, device=sharding)
            for a in out_avals)

    ctx = dict(sharded=sharded, in_names=in_names, out_names=out_names,
               out_avals=out_avals, sharding=sharding, mkzeros=_mkzeros)
    _EXEC_CACHE[key] = ctx
    return ctx


def run(inputs, S=S_FULL, dist=True, trace=False, **_):
    import jax
    maps = _host_shard(inputs, S, dist)
    if trace:
        nc = _get(S, dist)
        t0 = time.time()
        res = run_bass_kernel_spmd(nc, maps, core_ids=list(range(NCORES)),
                                   trace=trace)
        wall = time.time() - t0
        outs = res.results
        hc = np.stack([outs[r]["hc"] for r in range(NCORES)])
    else:
        ctx = _get_exec(S, dist)
        t0 = time.time()
        staged = [
            jax.device_put(
                np.concatenate([np.asarray(maps[c][nm])
                                for c in range(NCORES)], axis=0),
                ctx["sharding"])
            for nm in ctx["in_names"]]
        zeros = ctx["mkzeros"]()
        out_arrs = ctx["sharded"](*staged, *zeros)
        jax.block_until_ready(out_arrs)
        wall = time.time() - t0

        class _Res:
            exec_time_ns = None
            results = None
        res = _Res()
        i = ctx["out_names"].index("hc")
        hc = np.asarray(out_arrs[i]).reshape(
            NCORES, *ctx["out_avals"][i].shape)
    fwd = np.concatenate(
        [hc[r][0:B].astype(np.float32) for r in range(NCORES)], axis=2)
    bwd = np.concatenate(
        [hc[r][B:128].astype(np.float32) for r in range(NCORES)],
        axis=2)[:, ::-1]
    return (fwd, bwd), res, wall


def kernel(**inputs):
    (fwd, bwd), _, _ = run(inputs)
    return fwd.astype(np.float32), bwd.astype(np.float32)
